# revision 17
# baseline (speedup 1.0000x reference)
"""Trainium2 Bass kernel for the HexPlane-style decoder (nn_DecoderBase).

Math (B=1): six 3x3 SAME convs (64->16ch) + bias + ReLU + 2x nearest
upsample, channels-last, then broadcast Hadamard into
voxel[t, x, y, z, c] of shape [16, 64, 64, 32, 16] (f32, 128 MiB).

Sharding: X (64) split across 8 cores (8 x-values each; conv halos are
sliced host-side).  Per core the product factorizes as

  out[t,x,y,z,c] = M1[x,y,z,c] * ( ty[t,y,c] * Q[t,x,z,c] ),
  M1 = uxy*uxz*uyz,  Q = utx*utz.

Device layout: partition p = z0*64 + y (z = z0*16 + z1).  ty*Q is computed
by the TensorEngine as K=2 selector matmuls into PSUM (16 matmuls per t,
one per channel), so the VectorEngine runs a single fp32 tensor_tensor
pass (M1 * V) per t, overlapped with the 16 MiB/core output DMA.
"""

import numpy as np

T, XL, Y, Z, C = 16, 8, 64, 32, 16
NCORES = 8
CIN = 64

_CACHE = {}


def _build_program():
    from contextlib import ExitStack

    import concourse.bacc as bacc
    import concourse.bass as bass
    import concourse.mybir as mybir
    from concourse.tile import TileContext

    f32 = mybir.dt.float32
    AF = mybir.ActivationFunctionType
    MUL = mybir.AluOpType.mult
    AP = bass.AP

    nc = bacc.Bacc()
    ctx = ExitStack()

    # ---- external IO ----
    ein = lambda name, shape: nc.dram_tensor(name, shape, f32, kind="ExternalInput")
    img_xy = ein("img_xy", [CIN, 206])
    img_xz = ein("img_xz", [CIN, 110])
    img_yz = ein("img_yz", [CIN, 614])
    img_tx = ein("img_tx", [CIN, 62])
    img_ty = ein("img_ty", [CIN, 342])
    img_tz = ein("img_tz", [CIN, 182])
    wr = ein("wr", [CIN, 6, 3, 3, 16])      # (cin, plane, dy, dx, cout)
    b_flat = ein("b_flat", [1, 96])
    b_t = ein("b_t", [16, 6])
    ones_in = ein("ones_in", [1, 128])
    out_d = nc.dram_tensor("out", [T, XL, Y, Z, C], f32, kind="ExternalOutput")

    # ---- DRAM scratch: upsampled channels-last conv outputs ----
    dtx = nc.dram_tensor("dtx", [T, XL, C], f32)     # (t, x, c)
    dtz = nc.dram_tensor("dtz", [T, Z, C], f32)      # (t, z, c)
    dty = nc.dram_tensor("dty", [16, 8, 32], f32)    # (c, t', y') pre-upsample
    dxy = nc.dram_tensor("dxy", [XL, Y, C], f32)     # (x, y, c)
    dxz = nc.dram_tensor("dxz", [XL, Z, C], f32)     # (x, z, c)
    dyz = nc.dram_tensor("dyz", [Y, Z, C], f32)      # (y, z, c)
    qd = nc.dram_tensor("qd", [T, XL, Z, C], f32)    # (t, x, z, c)
    # raw conv-output dumps (flat [m*16], junk rows included)
    edump = {k: nc.dram_tensor(f"e_{k}", [m * 16], f32) for k, m in
             [("tx", 48), ("tz0", 72), ("tz1", 72), ("xy0", 68), ("xy1", 68),
              ("xz", 72), ("yz0", 126), ("yz1", 126), ("yz2", 126),
              ("yz3", 126), ("yz4", 72)]}

    with TileContext(nc) as tc:
        sb = lambda name, shape: ctx.enter_context(
            nc.sbuf_tensor(name, shape, f32))
        # inputs
        i_xy, i_xz, i_yz = (sb("i_xy", [CIN, 206]), sb("i_xz", [CIN, 110]),
                            sb("i_yz", [CIN, 614]))
        i_tx, i_ty, i_tz = (sb("i_tx", [CIN, 62]), sb("i_ty", [CIN, 342]),
                            sb("i_tz", [CIN, 182]))
        w_sb, bf_sb = sb("w_sb", [CIN, 864]), sb("bf_sb", [1, 96])
        bt_sb, on_sb = sb("bt_sb", [16, 6]), sb("on_sb", [1, 128])
        # voxel operands
        utx = sb("utx", [128, 16])        # p=(t,x): c
        utz = sb("utz", [128, 512])       # p=(t,x): (z,c)
        q_s = sb("q_s", [128, 512])       # p=(t,x): (z,c)
        uxy = sb("uxy", [128, 128])       # p=(z0,y): (x,c)
        uxz = sb("uxz", [128, 2048])      # p=(z0,y): (x,z1,c)
        uyz = sb("uyz", [128, 256])       # p=(z0,y): (z1,c)
        m1a = sb("m1a", [128, 2048])
        m1 = sb("m1", [128, 2048])
        ty_raw = sb("ty_raw", [1, 4096])   # (c', t', y') flat dty copy
        ty_all = sb("ty_all", [1, 8192])   # (t', c', y) with y upsampled

        # ---------- phase A: input loads ----------
        for dst, srca in ((i_xy, img_xy), (i_xz, img_xz), (i_yz, img_yz),
                          (i_tx, img_tx), (i_ty, img_ty), (i_tz, img_tz)):
            nc.sync.dma_start(dst[:], srca[:])
        nc.sync.dma_start(w_sb[:], wr[:].rearrange("a b c d e -> a (b c d e)"))
        nc.sync.dma_start(bf_sb[:], b_flat[:])
        nc.sync.dma_start(bt_sb[:], b_t[:])
        nc.sync.dma_start(on_sb[:], ones_in[:])

        # ---------- phase B: convolutions ----------
        def wslice(i, dy, dx):
            off = ((i * 3 + dy) * 3 + dx) * 16
            return w_sb[:, off:off + 16]

        conv_pool_cm = tc.tile_pool(name="convpsum", bufs=2, space="PSUM")
        conv_pool = conv_pool_cm.__enter__()

        conv_outs = {}

        def conv_spatial(i, img, fsz, wp, rows, row0, out_key, tag):
            # Full-width contiguous windows (stationary AP must be 1-D):
            # out flat m = r*wp + col; valid cols are 0..wp-3 (junk at edges).
            m = rows * wp
            psum = conv_pool.tile([m, 16], f32, name=f"cp_{tag}", tag="cp")
            for dy in range(3):
                for dx in range(3):
                    lhsT = AP(img, (row0 + dy) * wp + dx, [[fsz, CIN], [1, m]])
                    nc.tensor.matmul(psum, lhsT, wslice(i, dy, dx),
                                     start=(dy == 0 and dx == 0), stop=False)
            nc.tensor.matmul(psum, on_sb[:, :m],
                             bf_sb[:, i * 16:i * 16 + 16], start=False, stop=True)
            out_sb = sb(f"c_{tag}", [m, 16])
            nc.scalar.activation(out_sb[:], psum, AF.Relu)
            conv_outs[tag] = out_sb

        conv_spatial(3, i_tx, 62, 6, 8, 0, "tx", "tx")        # m=48
        conv_spatial(5, i_tz, 182, 18, 4, 0, "tz0", "tz0")    # m=72
        conv_spatial(5, i_tz, 182, 18, 4, 4, "tz1", "tz1")
        conv_spatial(0, i_xy, 206, 34, 2, 0, "xy0", "xy0")    # m=68
        conv_spatial(0, i_xy, 206, 34, 2, 2, "xy1", "xy1")
        conv_spatial(1, i_xz, 110, 18, 4, 0, "xz", "xz")      # m=72
        yz_rows = [(0, 7), (7, 7), (14, 7), (21, 7), (28, 4)]
        for bb, (r0, nr) in enumerate(yz_rows):
            conv_spatial(2, i_yz, 614, 18, nr, r0, f"yz{bb}", f"yz{bb}")

        # ty: cout-partition conv (W stationary, 1 free dim), full-width rhs
        psum_ty = conv_pool.tile([16, 272], f32, name="cp_ty", tag="cpty")
        for dy in range(3):
            for dx in range(3):
                rhs = AP(i_ty, dy * 34 + dx, [[342, CIN], [1, 272]])
                nc.tensor.matmul(psum_ty, wslice(4, dy, dx), rhs,
                                 start=(dy == 0 and dx == 0),
                                 stop=(dy == 2 and dx == 2))
        cty = sb("cty", [16, 272])  # (c, flat (t'*34 + y'), junk y'>31)
        nc.scalar.activation(cty[:], psum_ty, AF.Relu, bias=bt_sb[:, 4:5])

        conv_pool_cm.__exit__(None, None, None)

        # ---------- phase C: stage conv outputs to DRAM (upsample on store) ----
        # hop 1: SBUF conv tiles -> raw DRAM dumps (partition-stepping APs are
        # dim0-only on SBUF, so the layout fixup happens DRAM->DRAM in hop 2)
        dump_insts = {}
        for k in ["tx", "tz0", "tz1", "xy0", "xy1", "xz",
                  "yz0", "yz1", "yz2", "yz3", "yz4"]:
            dump_insts[k] = nc.sync.dma_start(edump[k][:], conv_outs[k][:])

        stage_insts = {}  # scratch-name -> [insts]

        def stage(key, srck, dst_ap, src_ap):
            inst = nc.sync.dma_start(dst_ap, src_ap)
            if srck is not None:
                bass._add_dep_helper(inst.ins, dump_insts[srck].ins,
                                     reason=f"dump {srck}")
            stage_insts.setdefault(key, []).append(inst)

        # dty[c, t', y'] <- cty valid cols (direct: free-dim junk only)
        stage("dty", None, dty[:].rearrange("a b c -> a (b c)"),
              AP(cty, 0, [[272, 16], [34, 8], [1, 32]]))
        # dtx[t, x, c] <- e_tx: flat m=(t'*6 + xloc)
        for rt in range(2):
            for rx in range(2):
                stage("dtx", "tx", AP(dtx, rt * XL * C + rx * C,
                                      [[2 * XL * C, 8], [2 * C, 4], [1, 16]]),
                      AP(edump["tx"], 0, [[96, 8], [16, 4], [1, 16]]))
        # dtz[t, z, c] <- e_tz{k}: m=(r*18 + z')
        for k in range(2):
            for rt in range(2):
                for rz in range(2):
                    stage("dtz", f"tz{k}",
                          AP(dtz, (8 * k + rt) * Z * C + rz * C,
                             [[2 * Z * C, 4], [2 * C, 16], [1, 16]]),
                          AP(edump[f"tz{k}"], 0, [[288, 4], [16, 16], [1, 16]]))
        # dxy[x, y, c] <- e_xy{k}: m=(r*34 + y')
        for k in range(2):
            for rx in range(2):
                for ry in range(2):
                    stage("dxy", f"xy{k}",
                          AP(dxy, (4 * k + rx) * Y * C + ry * C,
                             [[2 * Y * C, 2], [2 * C, 32], [1, 16]]),
                          AP(edump[f"xy{k}"], 0, [[544, 2], [16, 32], [1, 16]]))
        # dxz[x, z, c] <- e_xz
        for rx in range(2):
            for rz in range(2):
                stage("dxz", "xz", AP(dxz, rx * Z * C + rz * C,
                                      [[2 * Z * C, 4], [2 * C, 16], [1, 16]]),
                      AP(edump["xz"], 0, [[288, 4], [16, 16], [1, 16]]))
        # dyz[y, z, c] <- e_yz{bb}
        for bb, (r0, nr) in enumerate(yz_rows):
            for ry in range(2):
                for rz in range(2):
                    stage("dyz", f"yz{bb}",
                          AP(dyz, (2 * r0 + ry) * Z * C + rz * C,
                             [[2 * Z * C, nr], [2 * C, 16], [1, 16]]),
                          AP(edump[f"yz{bb}"], 0, [[288, nr], [16, 16], [1, 16]]))

        def after_stage(key, inst):
            # DRAM scratch RAW: make sure loads run after the staging stores
            for si in stage_insts[key]:
                bass._add_dep_helper(inst.ins, si.ins, reason=f"dram raw {key}")
            return inst

        # ---------- phase D: voxel operand loads, Q, M1 ----------
        after_stage("dtx", nc.sync.dma_start(
            utx[:], AP(dtx, 0, [[16, 128], [1, 16]])))
        after_stage("dtz", nc.sync.dma_start(
            utz[:], AP(dtz, 0, [[Z * C, 16], [0, 8], [1, Z * C]])))
        nc.vector.tensor_tensor(
            q_s[:], utz[:], AP(utx, 0, [[16, 128], [0, 32], [1, 16]]), MUL)
        q_store = nc.sync.dma_start(
            qd[:].rearrange("a b c d -> (a b) (c d)"), q_s[:])

        for z0 in range(2):
            after_stage("dxy", nc.sync.dma_start(
                uxy[z0 * 64:(z0 + 1) * 64, :],
                AP(dxy, 0, [[C, 64], [Y * C, 8], [1, 16]])))
            after_stage("dxz", nc.sync.dma_start(
                uxz[z0 * 64:(z0 + 1) * 64, :],
                AP(dxz, z0 * 16 * C, [[0, 64], [Z * C, 8], [1, 256]])))
        after_stage("dyz", nc.sync.dma_start(
            uyz[:], AP(dyz, 0, [[16 * C, 2], [Z * C, 64], [1, 256]])))

        after_stage("dty", nc.sync.dma_start(
            ty_raw[:], dty[:]))
        for tp in range(8):
            # ty_all[(t', c', y)] = ty_raw[(c', t', y//2)]
            nc.scalar.activation(
                AP(ty_all, tp * 1024, [[8192, 1], [1, 1024]]),
                AP(ty_raw, tp * 32, [[4096, 1], [256, 16], [1, 32], [0, 2]]),
                AF.Copy)

        nc.vector.tensor_tensor(
            m1a[:], uxz[:], AP(uyz, 0, [[256, 128], [0, 8], [1, 256]]), MUL)
        nc.vector.tensor_tensor(
            m1[:], m1a[:], AP(uxy, 0, [[128, 128], [16, 8], [0, 16], [1, 16]]),
            MUL)

        # ---------- phase E: per-t voxel ----------
        from contextlib import ExitStack as _ES
        pool_ctx = _ES()
        qz_pool = pool_ctx.enter_context(tc.tile_pool(name="qz", bufs=3))
        v_pool = pool_ctx.enter_context(
            tc.tile_pool(name="vps", bufs=2, space="PSUM"))
        out_pool = pool_ctx.enter_context(tc.tile_pool(name="outsb", bufs=3))

        for t in range(T):
            # Q rows for the two z0 halves: [1, (x, z1, c)]
            qzh = []
            for z0 in range(2):
                qz = qz_pool.tile([1, 2048], f32, name=f"qz{z0}", tag=f"qz{z0}")
                bass._add_dep_helper(
                    nc.sync.dma_start(
                        qz, AP(qd, t * XL * Z * C + z0 * 16 * C,
                               [[Z * C, 8], [1, 256]])).ins,
                    q_store.ins, reason="dram raw qd")
                qzh.append(qz)

            # V[p=(z0,y), (c', x, z1)] = ty[t,y,c'] * Q[t,x,z,c'] via K=1
            # outer-product matmuls, one per (z0 half, channel)
            v = v_pool.tile([128, 2048], f32, name="v", tag="v")
            vp = v.ap[0][0]
            for z0 in range(2):
                for cp in range(16):
                    lhsT = AP(ty_all, (t // 2) * 1024 + cp * 64,
                              [[8192, 1], [1, 64]])
                    rhs = AP(qzh[z0].tensor, qzh[z0].offset + cp,
                             [[2048, 1], [256, 8], [16, 16]])
                    nc.tensor.matmul(
                        v[z0 * 64:(z0 + 1) * 64, cp * 128:(cp + 1) * 128],
                        lhsT, rhs, start=True, stop=True)

            o = out_pool.tile([128, 2048], f32, name="o", tag="o")
            op = o.ap[0][0]
            nc.vector.tensor_tensor(
                AP(o.tensor, o.offset, [[op, 128], [256, 8], [16, 16], [1, 16]]),
                AP(m1, 0, [[2048, 128], [256, 8], [16, 16], [1, 16]]),
                AP(v.tensor, v.offset, [[vp, 128], [16, 8], [1, 16], [128, 16]]),
                MUL)
            for z0 in range(2):
                dst = AP(out_d, t * XL * Y * Z * C + z0 * 16 * C,
                         [[Z * C, 64], [Y * Z * C, 8], [1, 256]])
                nc.sync.dma_start(dst, o[z0 * 64:(z0 + 1) * 64, :])

        pool_ctx.close()

    nc.compile()
    return nc, ctx


def _prep_inputs(plane_xy, plane_xz, plane_yz, plane_tx, plane_ty, plane_tz, W, b):
    """Host-side slicing/padding/transposition. Returns per-core input maps."""
    f32 = np.float32
    xy = np.asarray(plane_xy, f32)[0]  # [64, X'32, Y'32]
    xz = np.asarray(plane_xz, f32)[0]  # [64, X'32, Z'16]
    yz = np.asarray(plane_yz, f32)[0]  # [64, Y'32, Z'16]
    tx = np.asarray(plane_tx, f32)[0]  # [64, T'8,  X'32]
    ty = np.asarray(plane_ty, f32)[0]  # [64, T'8,  Y'32]
    tz = np.asarray(plane_tz, f32)[0]  # [64, T'8,  Z'16]
    W = np.asarray(W, f32)             # [6, 16, 64, 3, 3]
    b = np.asarray(b, f32)             # [6, 16]

    wr = np.ascontiguousarray(W.transpose(2, 0, 3, 4, 1))  # (ci, i, dy, dx, co)
    b_flat = np.ascontiguousarray(b.reshape(1, 96))
    b_t = np.ascontiguousarray(b.T)
    ones = np.ones((1, 128), f32)

    def flat2(p):
        q = p.reshape(p.shape[0], -1)
        return np.ascontiguousarray(
            np.pad(q, ((0, 0), (0, 2))))

    img_yz = flat2(np.pad(yz, ((0, 0), (1, 1), (1, 1))))
    img_ty = flat2(np.pad(ty, ((0, 0), (1, 1), (1, 1))))
    img_tz = flat2(np.pad(tz, ((0, 0), (1, 1), (1, 1))))

    def row_halo(p, x0h):
        out = np.zeros((p.shape[0], 6, p.shape[2]), f32)
        lo = x0h - 1
        s0, s1 = max(lo, 0), min(lo + 6, p.shape[1])
        out[:, s0 - lo:s0 - lo + (s1 - s0), :] = p[:, s0:s1, :]
        return out

    def col_halo(p, x0h):
        out = np.zeros((p.shape[0], p.shape[1], 6), f32)
        lo = x0h - 1
        s0, s1 = max(lo, 0), min(lo + 6, p.shape[2])
        out[:, :, s0 - lo:s0 - lo + (s1 - s0)] = p[:, :, s0:s1]
        return out

    in_maps = []
    for k in range(NCORES):
        x0h = 4 * k
        in_maps.append({
            "img_xy": flat2(np.pad(row_halo(xy, x0h), ((0, 0), (0, 0), (1, 1)))),
            "img_xz": flat2(np.pad(row_halo(xz, x0h), ((0, 0), (0, 0), (1, 1)))),
            "img_yz": img_yz,
            "img_tx": flat2(np.pad(col_halo(tx, x0h), ((0, 0), (1, 1), (0, 0)))),
            "img_ty": img_ty,
            "img_tz": img_tz,
            "wr": wr,
            "b_flat": b_flat,
            "b_t": b_t,
            "ones_in": ones,
        })
    return in_maps


def kernel(plane_xy, plane_xz, plane_yz, plane_tx, plane_ty, plane_tz, W, b):
    from concourse.bass_utils import run_bass_kernel_spmd

    if "nc" not in _CACHE:
        _CACHE["nc"], _CACHE["ctx"] = _build_program()
    nc = _CACHE["nc"]

    in_maps = _prep_inputs(plane_xy, plane_xz, plane_yz, plane_tx, plane_ty,
                           plane_tz, W, b)
    res = run_bass_kernel_spmd(nc, in_maps, list(range(NCORES)))
    slices = [res.results[k]["out"] for k in range(NCORES)]
    full = np.concatenate(slices, axis=1)  # [T, 64, Y, Z, C]
    return full[None].astype(np.float32)


# revision 19
# speedup vs baseline: 1.3093x; 1.3093x over previous
"""Trainium2 Bass kernel for the HexPlane-style decoder (nn_DecoderBase).

Math (B=1): six 3x3 SAME convs (64->16ch) + bias + ReLU + 2x nearest
upsample, channels-last, then broadcast Hadamard into
voxel[t, x, y, z, c] of shape [16, 64, 64, 32, 16] (f32, 128 MiB).

Sharding: X (64) split across 8 cores (8 x-values each; conv halos are
sliced host-side).  Per core the product factorizes as

  out[t,x,y,z,c] = M1[x,y,z,c] * ( ty[t,y,c] * Q[t,x,z,c] ),
  M1 = uxy*uxz*uyz,  Q = utx*utz.

Device layout: partition p = z0*64 + y (z = z0*16 + z1).  ty*Q is computed
by the TensorEngine as K=2 selector matmuls into PSUM (16 matmuls per t,
one per channel), so the VectorEngine runs a single fp32 tensor_tensor
pass (M1 * V) per t, overlapped with the 16 MiB/core output DMA.
"""

import numpy as np

T, XL, Y, Z, C = 16, 8, 64, 32, 16
NCORES = 8
CIN = 64

_CACHE = {}


def _build_program():
    from contextlib import ExitStack

    import concourse.bacc as bacc
    import concourse.bass as bass
    import concourse.mybir as mybir
    from concourse.tile import TileContext

    f32 = mybir.dt.float32
    bf16 = mybir.dt.bfloat16
    AF = mybir.ActivationFunctionType
    MUL = mybir.AluOpType.mult
    AP = bass.AP

    nc = bacc.Bacc()
    ctx = ExitStack()

    # ---- external IO ----
    ein = lambda name, shape: nc.dram_tensor(name, shape, f32, kind="ExternalInput")
    img_xy = ein("img_xy", [CIN, 206])
    img_xz = ein("img_xz", [CIN, 110])
    img_yz = ein("img_yz", [CIN, 614])
    img_tx = ein("img_tx", [CIN, 62])
    img_ty = ein("img_ty", [CIN, 342])
    img_tz = ein("img_tz", [CIN, 182])
    wr = ein("wr", [CIN, 6, 3, 3, 16])      # (cin, plane, dy, dx, cout)
    b_flat = ein("b_flat", [1, 96])
    b_t = ein("b_t", [16, 6])
    ones_in = ein("ones_in", [1, 128])
    out_d = nc.dram_tensor("out", [T, XL, Y, Z, C], f32, kind="ExternalOutput")

    # ---- DRAM scratch: upsampled channels-last conv outputs ----
    dtx = nc.dram_tensor("dtx", [T, XL, C], f32)     # (t, x, c)
    dtz = nc.dram_tensor("dtz", [T, Z, C], f32)      # (t, z, c)
    dty = nc.dram_tensor("dty", [16, 8, 32], f32)    # (c, t', y') pre-upsample
    dxy = nc.dram_tensor("dxy", [XL, Y, C], f32)     # (x, y, c)
    dxz = nc.dram_tensor("dxz", [XL, Z, C], f32)     # (x, z, c)
    dyz = nc.dram_tensor("dyz", [Y, Z, C], f32)      # (y, z, c)
    qd = nc.dram_tensor("qd", [T * XL * Z * C + 16], bf16)  # (t,x,z,c) flat +pad
    # raw conv-output dumps (flat [m*16], junk rows included)
    edump = {k: nc.dram_tensor(f"e_{k}", [m * 16], f32) for k, m in
             [("tx", 48), ("tz0", 72), ("tz1", 72), ("xy0", 68), ("xy1", 68),
              ("xz", 72), ("yz0", 126), ("yz1", 126), ("yz2", 126),
              ("yz3", 126), ("yz4", 72)]}

    with TileContext(nc) as tc:
        sb = lambda name, shape: ctx.enter_context(
            nc.sbuf_tensor(name, shape, f32))
        # inputs
        i_xy, i_xz, i_yz = (sb("i_xy", [CIN, 206]), sb("i_xz", [CIN, 110]),
                            sb("i_yz", [CIN, 614]))
        i_tx, i_ty, i_tz = (sb("i_tx", [CIN, 62]), sb("i_ty", [CIN, 342]),
                            sb("i_tz", [CIN, 182]))
        w_sb, bf_sb = sb("w_sb", [CIN, 864]), sb("bf_sb", [1, 96])
        bt_sb, on_sb = sb("bt_sb", [16, 6]), sb("on_sb", [1, 128])
        # voxel operands
        utx = sb("utx", [128, 16])        # p=(t,x): c
        utz = sb("utz", [128, 512])       # p=(t,x): (z,c)
        q_s = ctx.enter_context(nc.sbuf_tensor("q_s", [128, 512], bf16))
        uxy = sb("uxy", [128, 128])       # p=(z0,y): (x,c)
        uxz = sb("uxz", [128, 2048])      # p=(z0,y): (x,z1,c)
        uyz = sb("uyz", [128, 256])       # p=(z0,y): (z1,c)
        m1a = sb("m1a", [128, 2048])
        m1 = sb("m1", [128, 2048])
        ty_raw = sb("ty_raw", [1, 4096])   # (c', t', y') flat dty copy
        ty_all = ctx.enter_context(nc.sbuf_tensor("ty_all", [1, 8192], bf16))

        # ---------- phase A: input loads ----------
        for dst, srca in ((i_xy, img_xy), (i_xz, img_xz), (i_yz, img_yz),
                          (i_tx, img_tx), (i_ty, img_ty), (i_tz, img_tz)):
            nc.sync.dma_start(dst[:], srca[:])
        nc.sync.dma_start(w_sb[:], wr[:].rearrange("a b c d e -> a (b c d e)"))
        nc.sync.dma_start(bf_sb[:], b_flat[:])
        nc.sync.dma_start(bt_sb[:], b_t[:])
        nc.sync.dma_start(on_sb[:], ones_in[:])

        # ---------- phase B: convolutions ----------
        def wslice(i, dy, dx):
            off = ((i * 3 + dy) * 3 + dx) * 16
            return w_sb[:, off:off + 16]

        conv_pool_cm = tc.tile_pool(name="convpsum", bufs=2, space="PSUM")
        conv_pool = conv_pool_cm.__enter__()

        conv_outs = {}

        def conv_spatial(i, img, fsz, wp, rows, row0, out_key, tag):
            # Full-width contiguous windows (stationary AP must be 1-D):
            # out flat m = r*wp + col; valid cols are 0..wp-3 (junk at edges).
            m = rows * wp
            psum = conv_pool.tile([m, 16], f32, name=f"cp_{tag}", tag="cp")
            for dy in range(3):
                for dx in range(3):
                    lhsT = AP(img, (row0 + dy) * wp + dx, [[fsz, CIN], [1, m]])
                    nc.tensor.matmul(psum, lhsT, wslice(i, dy, dx),
                                     start=(dy == 0 and dx == 0), stop=False)
            nc.tensor.matmul(psum, on_sb[:, :m],
                             bf_sb[:, i * 16:i * 16 + 16], start=False, stop=True)
            out_sb = sb(f"c_{tag}", [m, 16])
            nc.scalar.activation(out_sb[:], psum, AF.Relu)
            conv_outs[tag] = out_sb

        conv_spatial(3, i_tx, 62, 6, 8, 0, "tx", "tx")        # m=48
        conv_spatial(5, i_tz, 182, 18, 4, 0, "tz0", "tz0")    # m=72
        conv_spatial(5, i_tz, 182, 18, 4, 4, "tz1", "tz1")
        conv_spatial(0, i_xy, 206, 34, 2, 0, "xy0", "xy0")    # m=68
        conv_spatial(0, i_xy, 206, 34, 2, 2, "xy1", "xy1")
        conv_spatial(1, i_xz, 110, 18, 4, 0, "xz", "xz")      # m=72
        yz_rows = [(0, 7), (7, 7), (14, 7), (21, 7), (28, 4)]
        for bb, (r0, nr) in enumerate(yz_rows):
            conv_spatial(2, i_yz, 614, 18, nr, r0, f"yz{bb}", f"yz{bb}")

        # ty: cout-partition conv (W stationary, 1 free dim), full-width rhs
        psum_ty = conv_pool.tile([16, 272], f32, name="cp_ty", tag="cpty")
        for dy in range(3):
            for dx in range(3):
                rhs = AP(i_ty, dy * 34 + dx, [[342, CIN], [1, 272]])
                nc.tensor.matmul(psum_ty, wslice(4, dy, dx), rhs,
                                 start=(dy == 0 and dx == 0),
                                 stop=(dy == 2 and dx == 2))
        cty = sb("cty", [16, 272])  # (c, flat (t'*34 + y'), junk y'>31)
        nc.scalar.activation(cty[:], psum_ty, AF.Relu, bias=bt_sb[:, 4:5])

        conv_pool_cm.__exit__(None, None, None)

        # ---------- phase C: stage conv outputs to DRAM (upsample on store) ----
        # hop 1: SBUF conv tiles -> raw DRAM dumps (partition-stepping APs are
        # dim0-only on SBUF, so the layout fixup happens DRAM->DRAM in hop 2)
        dump_insts = {}
        for k in ["tx", "tz0", "tz1", "xy0", "xy1", "xz",
                  "yz0", "yz1", "yz2", "yz3", "yz4"]:
            dump_insts[k] = nc.sync.dma_start(edump[k][:], conv_outs[k][:])

        stage_insts = {}  # scratch-name -> [insts]

        def stage(key, srck, dst_ap, src_ap):
            inst = nc.sync.dma_start(dst_ap, src_ap)
            if srck is not None:
                bass._add_dep_helper(inst.ins, dump_insts[srck].ins,
                                     reason=f"dump {srck}")
            stage_insts.setdefault(key, []).append(inst)

        # dty[c, t', y'] <- cty valid cols (direct: free-dim junk only)
        stage("dty", None, dty[:].rearrange("a b c -> a (b c)"),
              AP(cty, 0, [[272, 16], [34, 8], [1, 32]]))
        # dtx[t, x, c] <- e_tx: flat m=(t'*6 + xloc)
        for rt in range(2):
            for rx in range(2):
                stage("dtx", "tx", AP(dtx, rt * XL * C + rx * C,
                                      [[2 * XL * C, 8], [2 * C, 4], [1, 16]]),
                      AP(edump["tx"], 0, [[96, 8], [16, 4], [1, 16]]))
        # dtz[t, z, c] <- e_tz{k}: m=(r*18 + z')
        for k in range(2):
            for rt in range(2):
                for rz in range(2):
                    stage("dtz", f"tz{k}",
                          AP(dtz, (8 * k + rt) * Z * C + rz * C,
                             [[2 * Z * C, 4], [2 * C, 16], [1, 16]]),
                          AP(edump[f"tz{k}"], 0, [[288, 4], [16, 16], [1, 16]]))
        # dxy[x, y, c] <- e_xy{k}: m=(r*34 + y')
        for k in range(2):
            for rx in range(2):
                for ry in range(2):
                    stage("dxy", f"xy{k}",
                          AP(dxy, (4 * k + rx) * Y * C + ry * C,
                             [[2 * Y * C, 2], [2 * C, 32], [1, 16]]),
                          AP(edump[f"xy{k}"], 0, [[544, 2], [16, 32], [1, 16]]))
        # dxz[x, z, c] <- e_xz
        for rx in range(2):
            for rz in range(2):
                stage("dxz", "xz", AP(dxz, rx * Z * C + rz * C,
                                      [[2 * Z * C, 4], [2 * C, 16], [1, 16]]),
                      AP(edump["xz"], 0, [[288, 4], [16, 16], [1, 16]]))
        # dyz[y, z, c] <- e_yz{bb}
        for bb, (r0, nr) in enumerate(yz_rows):
            for ry in range(2):
                for rz in range(2):
                    stage("dyz", f"yz{bb}",
                          AP(dyz, (2 * r0 + ry) * Z * C + rz * C,
                             [[2 * Z * C, nr], [2 * C, 16], [1, 16]]),
                          AP(edump[f"yz{bb}"], 0, [[288, nr], [16, 16], [1, 16]]))

        def after_stage(key, inst):
            # DRAM scratch RAW: make sure loads run after the staging stores
            for si in stage_insts[key]:
                bass._add_dep_helper(inst.ins, si.ins, reason=f"dram raw {key}")
            return inst

        # ---------- phase D: voxel operand loads, Q, M1 ----------
        after_stage("dtx", nc.sync.dma_start(
            utx[:], AP(dtx, 0, [[16, 128], [1, 16]])))
        after_stage("dtz", nc.sync.dma_start(
            utz[:], AP(dtz, 0, [[Z * C, 16], [0, 8], [1, Z * C]])))
        nc.vector.tensor_tensor(
            q_s[:], utz[:], AP(utx, 0, [[16, 128], [0, 32], [1, 16]]), MUL)
        q_store = nc.sync.dma_start(
            AP(qd, 0, [[512, 128], [1, 512]]), q_s[:])

        for z0 in range(2):
            after_stage("dxy", nc.sync.dma_start(
                uxy[z0 * 64:(z0 + 1) * 64, :],
                AP(dxy, 0, [[C, 64], [Y * C, 8], [1, 16]])))
            after_stage("dxz", nc.sync.dma_start(
                uxz[z0 * 64:(z0 + 1) * 64, :],
                AP(dxz, z0 * 16 * C, [[0, 64], [Z * C, 8], [1, 256]])))
        after_stage("dyz", nc.sync.dma_start(
            uyz[:], AP(dyz, 0, [[16 * C, 2], [Z * C, 64], [1, 256]])))

        after_stage("dty", nc.sync.dma_start(
            ty_raw[:], dty[:]))
        for tp in range(8):
            # ty_all[(t', c', y)] = ty_raw[(c', t', y//2)]
            nc.scalar.activation(
                AP(ty_all, tp * 1024, [[8192, 1], [1, 1024]]),
                AP(ty_raw, tp * 32, [[4096, 1], [256, 16], [1, 32], [0, 2]]),
                AF.Copy)

        nc.vector.tensor_tensor(
            m1a[:], uxz[:], AP(uyz, 0, [[256, 128], [0, 8], [1, 256]]), MUL)
        nc.vector.tensor_tensor(
            m1[:], m1a[:], AP(uxy, 0, [[128, 128], [16, 8], [0, 16], [1, 16]]),
            MUL)

        # ---------- phase E: per-t voxel ----------
        from contextlib import ExitStack as _ES
        pool_ctx = _ES()
        qz_pool = pool_ctx.enter_context(tc.tile_pool(name="qz", bufs=3))
        v_pool = pool_ctx.enter_context(
            tc.tile_pool(name="vps", bufs=2, space="PSUM"))
        out_pool = pool_ctx.enter_context(tc.tile_pool(name="outsb", bufs=3))

        for t in range(T):
            # Q rows for the two z0 halves: [1, (x, z1, c)]; the "o" copy is
            # shifted one element so odd-channel slices read 4B-aligned bases
            qzh, qzo = [], []
            for z0 in range(2):
                qz = qz_pool.tile([1, 2048], bf16, name=f"qz{z0}", tag=f"qz{z0}")
                bass._add_dep_helper(
                    nc.sync.dma_start(
                        qz, AP(qd, t * XL * Z * C + z0 * 16 * C,
                               [[Z * C, 8], [1, 256]])).ins,
                    q_store.ins, reason="dram raw qd")
                qzh.append(qz)
                qo = qz_pool.tile([1, 2048], bf16, name=f"qo{z0}", tag=f"qo{z0}")
                bass._add_dep_helper(
                    nc.sync.dma_start(
                        qo, AP(qd, t * XL * Z * C + z0 * 16 * C + 1,
                               [[Z * C, 8], [1, 256]])).ins,
                    q_store.ins, reason="dram raw qd")
                qzo.append(qo)

            # V[p=(z0,y), (c', x, z1)] = ty[t,y,c'] * Q[t,x,z,c'] via K=1
            # outer-product matmuls, one per (z0 half, channel)
            v = v_pool.tile([128, 2048], f32, name="v", tag="v")
            vp = v.ap[0][0]
            for z0 in range(2):
                for cp in range(16):
                    lhsT = AP(ty_all, (t // 2) * 1024 + cp * 64,
                              [[8192, 1], [1, 64]])
                    if cp % 2 == 0:
                        rhs = AP(qzh[z0].tensor, qzh[z0].offset + cp,
                                 [[2048, 1], [256, 8], [16, 16]])
                    else:
                        rhs = AP(qzo[z0].tensor, qzo[z0].offset + cp - 1,
                                 [[2048, 1], [256, 8], [16, 16]])
                    nc.tensor.matmul(
                        v[z0 * 64:(z0 + 1) * 64, cp * 128:(cp + 1) * 128],
                        lhsT, rhs, start=True, stop=True)

            o = out_pool.tile([128, 2048], f32, name="o", tag="o")
            op = o.ap[0][0]
            nc.vector.tensor_tensor(
                AP(o.tensor, o.offset, [[op, 128], [256, 8], [16, 16], [1, 16]]),
                AP(m1, 0, [[2048, 128], [256, 8], [16, 16], [1, 16]]),
                AP(v.tensor, v.offset, [[vp, 128], [16, 8], [1, 16], [128, 16]]),
                MUL)
            for z0 in range(2):
                dst = AP(out_d, t * XL * Y * Z * C + z0 * 16 * C,
                         [[Z * C, 64], [Y * Z * C, 8], [1, 256]])
                nc.sync.dma_start(dst, o[z0 * 64:(z0 + 1) * 64, :])

        pool_ctx.close()

    nc.compile()
    return nc, ctx


def _prep_inputs(plane_xy, plane_xz, plane_yz, plane_tx, plane_ty, plane_tz, W, b):
    """Host-side slicing/padding/transposition. Returns per-core input maps."""
    f32 = np.float32
    xy = np.asarray(plane_xy, f32)[0]  # [64, X'32, Y'32]
    xz = np.asarray(plane_xz, f32)[0]  # [64, X'32, Z'16]
    yz = np.asarray(plane_yz, f32)[0]  # [64, Y'32, Z'16]
    tx = np.asarray(plane_tx, f32)[0]  # [64, T'8,  X'32]
    ty = np.asarray(plane_ty, f32)[0]  # [64, T'8,  Y'32]
    tz = np.asarray(plane_tz, f32)[0]  # [64, T'8,  Z'16]
    W = np.asarray(W, f32)             # [6, 16, 64, 3, 3]
    b = np.asarray(b, f32)             # [6, 16]

    wr = np.ascontiguousarray(W.transpose(2, 0, 3, 4, 1))  # (ci, i, dy, dx, co)
    b_flat = np.ascontiguousarray(b.reshape(1, 96))
    b_t = np.ascontiguousarray(b.T)
    ones = np.ones((1, 128), f32)

    def flat2(p):
        q = p.reshape(p.shape[0], -1)
        return np.ascontiguousarray(
            np.pad(q, ((0, 0), (0, 2))))

    img_yz = flat2(np.pad(yz, ((0, 0), (1, 1), (1, 1))))
    img_ty = flat2(np.pad(ty, ((0, 0), (1, 1), (1, 1))))
    img_tz = flat2(np.pad(tz, ((0, 0), (1, 1), (1, 1))))

    def row_halo(p, x0h):
        out = np.zeros((p.shape[0], 6, p.shape[2]), f32)
        lo = x0h - 1
        s0, s1 = max(lo, 0), min(lo + 6, p.shape[1])
        out[:, s0 - lo:s0 - lo + (s1 - s0), :] = p[:, s0:s1, :]
        return out

    def col_halo(p, x0h):
        out = np.zeros((p.shape[0], p.shape[1], 6), f32)
        lo = x0h - 1
        s0, s1 = max(lo, 0), min(lo + 6, p.shape[2])
        out[:, :, s0 - lo:s0 - lo + (s1 - s0)] = p[:, :, s0:s1]
        return out

    in_maps = []
    for k in range(NCORES):
        x0h = 4 * k
        in_maps.append({
            "img_xy": flat2(np.pad(row_halo(xy, x0h), ((0, 0), (0, 0), (1, 1)))),
            "img_xz": flat2(np.pad(row_halo(xz, x0h), ((0, 0), (0, 0), (1, 1)))),
            "img_yz": img_yz,
            "img_tx": flat2(np.pad(col_halo(tx, x0h), ((0, 0), (1, 1), (0, 0)))),
            "img_ty": img_ty,
            "img_tz": img_tz,
            "wr": wr,
            "b_flat": b_flat,
            "b_t": b_t,
            "ones_in": ones,
        })
    return in_maps


def kernel(plane_xy, plane_xz, plane_yz, plane_tx, plane_ty, plane_tz, W, b):
    from concourse.bass_utils import run_bass_kernel_spmd

    if "nc" not in _CACHE:
        _CACHE["nc"], _CACHE["ctx"] = _build_program()
    nc = _CACHE["nc"]

    in_maps = _prep_inputs(plane_xy, plane_xz, plane_yz, plane_tx, plane_ty,
                           plane_tz, W, b)
    res = run_bass_kernel_spmd(nc, in_maps, list(range(NCORES)))
    slices = [res.results[k]["out"] for k in range(NCORES)]
    full = np.concatenate(slices, axis=1)  # [T, 64, Y, Z, C]
    return full[None].astype(np.float32)
